# revision 13
# baseline (speedup 1.0000x reference)
"""MFA per-component log-likelihood kernel for 8x TRN2 NeuronCores.

Math: out[n,k] = base[k] + sum_m (x_n . g_km)^2 + x_n . Wx_k + (x_n^2) . Wxx_k
with g/Wx/base from the Woodbury factorization (host-side, tiny).

Device strategy (per core, N_SHARD=2048 rows, output TRANSPOSED [K, n]):
  - Weights-stationary fp8 (e4m3) DoubleRow matmuls for the factor ("quad")
    columns: stationary = Gw block [128d, 2, 128cols], moving = x_fp8
    [128d, 2, 512n] -> psum y-block [128cols, 512n], 256-deep contraction
    per streamed column (2x fp32 MAC rate).
  - ScalarE squares psum with scale 1/64 into fp8 "sq" pairs.
  - Group-of-16 reduction on the PE: fp8 DoubleRow matmul against a
    constant 0/4 block-indicator matrix (S2), accumulating quad directly
    into the per-n-block accumulator psum bank [128k, 512n].
  - Linear term x.Wx (fp8 DoubleRow, same moving x) and x^2.Wxx (fp16 for
    precision -- fp8 wxx/xsq triples the output error) accumulate into the
    same bank, placed before the block's final group-sum so the PE never
    waits on the scalar engine.
  - DVE adds base (per-partition scalar) while copying acc psum -> SBUF
    fp16, DMA out per block (overlapped with the next block's matmuls).
    Host transposes [K,N] -> [N,K] and widens to fp32.

Sharding: rows N=16384 split across 8 cores; params replicated.
DMA: all tensors are stored PIECE-MAJOR in DRAM and SBUF so every piece
transfer is contiguous per partition (1-8KB runs; sub-KB strided runs
crawl at ~12GB/s on the DGE rings). Pieces ride the two hardware rings
(sync + scalar) ordered by first-use time; tiny constants and the block
outputs use the gpsimd software ring.
"""

import math

import numpy as np

K, D_FEAT, L_FAC, N = 128, 1024, 16, 16384
N_CORES = 8
N_SHARD = N // N_CORES            # 2048 rows per core
NB = N_SHARD // 512               # 4 moving blocks of 512 rows
J2 = 4                            # DoubleRow contraction chunks (256 each)
JJ = 8                            # fp16 xsq chunks (128 each)
CB = 16                           # 128-col blocks of factor columns
NPAIR = CB // 2                   # S2 pair matmuls per n-block
WPC = 4                           # wq pieces (512 factor cols each)
GCOLS = K * L_FAC                 # 2048 factor columns
SG = 32.0                         # Gw fp8 scale
SQ_SCALE = 1.0 / 64.0             # scalar: sq = (psum/64)^2 = y^2/4
S2_VAL = 4.0                      # un-scales sq in the group-sum matmul

_CACHE = {}


def _get_nc():
    if "nc" in _CACHE:
        return _CACHE["nc"]

    import concourse.bass as bass
    import concourse.tile as tile
    from concourse import bacc, mybir

    f32 = mybir.dt.float32
    f16 = mybir.dt.float16
    f8 = mybir.dt.float8e4
    DR = mybir.MatmulPerfMode.DoubleRow
    nc = bacc.Bacc("TRN2", target_bir_lowering=False, debug=False,
                   num_devices=N_CORES)

    xq = nc.dram_tensor("xq", [128, NB, J2, 2, 512], f8, kind="ExternalInput").ap()
    xsqh = nc.dram_tensor("xsqh", [128, NB, JJ, 512], f16, kind="ExternalInput").ap()
    wq = nc.dram_tensor("wq", [128, WPC, J2, 2, 512], f8, kind="ExternalInput").ap()
    wx = nc.dram_tensor("wx", [128, J2, 2, K], f8, kind="ExternalInput").ap()
    wxxh = nc.dram_tensor("wxxh", [128, JJ, K], f16, kind="ExternalInput").ap()
    s2 = nc.dram_tensor("s2", [128, NPAIR, 2, K], f8, kind="ExternalInput").ap()
    bs = nc.dram_tensor("bs", [128, 1], f32, kind="ExternalInput").ap()
    outT = nc.dram_tensor("outT", [128, N_SHARD], f16, kind="ExternalOutput").ap()

    with tile.TileContext(nc) as tc:
        with (
            tc.tile_pool(name="singles", bufs=1) as singles,
            tc.tile_pool(name="sqpool", bufs=4) as sqpool,
            tc.tile_pool(name="upool", bufs=2) as upool,
            tc.tile_pool(name="qp", bufs=6, space="PSUM") as qp,
            tc.tile_pool(name="accp", bufs=2, space="PSUM") as accp,
        ):
            wq_s = singles.tile([128, WPC, J2, 2, 512], f8, tag="wq")
            wx_s = singles.tile([128, J2, 2, K], f8, tag="wx")
            xq_s = singles.tile([128, NB, J2, 2, 512], f8, tag="xq")
            xs_s = singles.tile([128, NB, JJ, 512], f16, tag="xs")
            wxx_s = singles.tile([128, JJ, K], f16, tag="wxx")
            s2_s = singles.tile([128, NPAIR, 2, K], f8, tag="s2")
            bs_s = singles.tile([128, 1], f32, tag="bs")
            dmy = singles.tile([128, 2, 512], f8, tag="dmy")

            nc.vector.memset(dmy, 0)

            # The SCALAR queue carries NO input DMAs: a DMA issue that has
            # to wait on a recycled DGE semaphore blocks the queue, and the
            # ACTIVATE squares behind it would stall every group-sum.
            # gpsimd software ring: s2 first (needed by the first group-sum
            # ~10us in), then the other tiny constants.
            nc.gpsimd.dma_start(out=s2_s, in_=s2)
            nc.gpsimd.dma_start(out=bs_s, in_=bs)
            nc.gpsimd.dma_start(out=wxx_s, in_=wxxh)

            # Early pieces split by PARTITION HALVES across the sync hw ring
            # and the gpsimd ring: piece latency is descriptor-serialization
            # bound (~30ns x 128 partition runs), so 64-partition halves on
            # two rings land in half the time. Order is first-use time.
            early = [(wq_s[:, 0], wq[:, 0]), (xq_s[:, 0], xq[:, 0]),
                     (wq_s[:, 1], wq[:, 1]), (xq_s[:, 1], xq[:, 1]),
                     (wq_s[:, 2], wq[:, 2]), (wq_s[:, 3], wq[:, 3]),
                     (wx_s, wx)]
            for dst, src in early:
                nc.sync.dma_start(out=dst[0:64], in_=src[0:64])
                nc.gpsimd.dma_start(out=dst[64:128], in_=src[64:128])
            # Remaining pieces are off the critical path; whole pieces.
            nc.sync.dma_start(out=xs_s[:, 0], in_=xsqh[:, 0])
            nc.gpsimd.dma_start(out=xq_s[:, 2], in_=xq[:, 2])
            nc.sync.dma_start(out=xs_s[:, 1], in_=xsqh[:, 1])
            nc.gpsimd.dma_start(out=xq_s[:, 3], in_=xq[:, 3])
            nc.sync.dma_start(out=xs_s[:, 2], in_=xsqh[:, 2])
            nc.gpsimd.dma_start(out=xs_s[:, 3], in_=xsqh[:, 3])

            # Dummy matmuls ramp the PE p-state while the first DMAs land.
            for _ in range(5):
                wrm = qp.tile([128, 512], f32, tag="q")
                nc.tensor.matmul(wrm, dmy[:, :, 0:128], dmy,
                                 start=True, stop=True, perf_mode=DR)

            state = {}

            def emit_pairs(nb, pairs):
                st = state.setdefault(nb, {"acc": None, "pending": None})
                if st["acc"] is None:
                    acc_t = accp.tile([128, 512], f32, tag="acc")
                    st["acc"] = acc_t
                acc = st["acc"]
                for pair in pairs:
                    sq_t = sqpool.tile([128, 2, 512], f8, tag="sq")
                    for r in range(2):
                        cb = 2 * pair + r
                        q = qp.tile([128, 512], f32, tag="q")
                        for j2 in range(J2):
                            nc.tensor.matmul(
                                q,
                                wq_s[:, cb // 4, j2, :,
                                     (cb % 4) * 128:(cb % 4 + 1) * 128],
                                xq_s[:, nb, j2],
                                start=(j2 == 0), stop=(j2 == J2 - 1),
                                perf_mode=DR)
                        nc.scalar.activation(
                            sq_t[:, r, :], q,
                            mybir.ActivationFunctionType.Square,
                            scale=SQ_SCALE)
                    # defer the group-sum one pair so the square can finish
                    if st["pending"] is not None:
                        p_pair, p_sq = st["pending"]
                        nc.tensor.matmul(acc, s2_s[:, p_pair], p_sq,
                                         start=(p_pair == 0), stop=False,
                                         perf_mode=DR)
                    st["pending"] = (pair, sq_t)

            def emit_block_tail(nb, split_out):
                st = state[nb]
                acc = st["acc"]
                nbs = slice(nb * 512, (nb + 1) * 512)
                # linear terms x.Wx and x^2.Wxx run while the scalar engine
                # finishes the last pair's squares
                for j2 in range(J2):
                    nc.tensor.matmul(acc, wx_s[:, j2], xq_s[:, nb, j2],
                                     start=False, stop=False, perf_mode=DR)
                for jj in range(JJ):
                    nc.tensor.matmul(acc, wxx_s[:, jj], xs_s[:, nb, jj],
                                     start=False, stop=False)
                p_pair, p_sq = st["pending"]
                nc.tensor.matmul(acc, s2_s[:, p_pair], p_sq,
                                 start=False, stop=True, perf_mode=DR)
                # block output immediately: overlaps later matmuls. The last
                # block splits across the two (by then idle) hw rings.
                if not split_out:
                    u = upool.tile([128, 512], f16, tag="u")
                    nc.vector.tensor_scalar_add(out=u, in0=acc, scalar1=bs_s)
                    nc.gpsimd.dma_start(out=outT[:, nbs], in_=u)
                else:
                    # partition-halved across the (by now idle) hw rings
                    u = upool.tile([128, 512], f16, tag="u")
                    nc.vector.tensor_scalar_add(out=u, in0=acc, scalar1=bs_s)
                    nc.sync.dma_start(out=outT[0:64, nbs], in_=u[0:64])
                    nc.scalar.dma_start(out=outT[64:128, nbs], in_=u[64:128])

            # Block heads of nb0/nb1 are interleaved so the PE always has
            # work that matches what the DMA rings have delivered (xq piece
            # arrival alternates with wq piece arrival early on).
            emit_pairs(0, [0, 1])
            emit_pairs(1, [0, 1])
            emit_pairs(0, range(2, NPAIR))
            emit_block_tail(0, split_out=False)
            emit_pairs(1, range(2, NPAIR))
            emit_block_tail(1, split_out=False)
            emit_pairs(2, range(NPAIR))
            emit_block_tail(2, split_out=False)
            emit_pairs(3, range(NPAIR))
            emit_block_tail(3, split_out=True)

    nc.finalize()
    _CACHE["nc"] = nc
    return nc


def _host_params(PI, MU, A, D):
    import ml_dtypes
    FP8 = ml_dtypes.float8_e4m3

    PI64 = PI.astype(np.float64)
    MU64 = MU.astype(np.float64)
    A64 = A.astype(np.float64)
    D64 = D.astype(np.float64)

    iD = D64 ** -2.0                                   # (K, d)
    iDA = iD[:, :, None] * A64                         # (K, d, l)
    Lm = np.eye(L_FAC)[None] + np.einsum("kdl,kdm->klm", A64, iDA)
    iL = np.linalg.inv(Lm)
    C = np.linalg.cholesky(iL)                         # iL = C C^T
    s = 1.0 / math.sqrt(2.0)
    G = np.einsum("kdl,klm->kdm", iDA, C) * s          # (K, d, l)
    b = np.einsum("kd,kdl->kl", MU64, iDA)             # (K, l)
    h = np.einsum("kl,klm->km", b, C) * s              # (K, l)

    Gw = G.transpose(1, 0, 2).reshape(D_FEAT, GCOLS)   # col k*16+m
    Wx = (iD * MU64).T - 2.0 * np.einsum("kdm,km->kd", G, h).T
    Wxx = -0.5 * iD.T                                  # (d, K)

    det_L = np.linalg.slogdet(Lm)[1]
    log_det_sigma = det_L - np.sum(np.log(iD), axis=1)
    c1 = np.sum(iD * MU64 * MU64, axis=1)
    hsq = np.sum(h * h, axis=1)
    base = PI64 - 0.5 * (D_FEAT * math.log(2.0 * math.pi)
                         + log_det_sigma + c1) + hsq

    # wq piece-major: [128, WPC, J2, 2, 512], piece pc = factor cols
    # [pc*512, (pc+1)*512); d index = j2*256 + r*128 + p.
    gw8 = (Gw * SG).astype(np.float32).astype(FP8)     # (d, 2048)
    wq = np.ascontiguousarray(
        gw8.reshape(J2, 2, 128, WPC, 512).transpose(2, 3, 0, 1, 4))
    wx8 = np.ascontiguousarray(
        Wx.astype(np.float32).astype(FP8)
        .reshape(J2, 2, 128, K).transpose(2, 0, 1, 3))

    wxxh = np.ascontiguousarray(
        Wxx.astype(np.float32).reshape(JJ, 128, K).transpose(1, 0, 2)
    ).astype(np.float16)

    s2 = np.zeros((128, NPAIR, 2, K), dtype=np.float32)
    p_idx = np.arange(128)
    for pair in range(NPAIR):
        for r in range(2):
            cb = 2 * pair + r
            s2[p_idx, pair, r, cb * 8 + p_idx // 16] = S2_VAL
    s2 = s2.astype(FP8)

    bs_np = np.ascontiguousarray(base.astype(np.float32).reshape(128, 1))
    return wq, wx8, wxxh, s2, bs_np


def kernel(x, PI, MU, A, D, _trace=False):
    from concourse.bass_utils import run_bass_kernel_spmd
    import ml_dtypes
    FP8 = ml_dtypes.float8_e4m3

    x = np.asarray(x, dtype=np.float32)
    wq, wx8, wxxh, s2, bs_np = _host_params(
        np.asarray(PI), np.asarray(MU), np.asarray(A), np.asarray(D))

    in_maps = []
    for c in range(N_CORES):
        xs = x[c * N_SHARD:(c + 1) * N_SHARD]          # (2048, 1024)
        x8t = xs.astype(FP8).T                         # (1024, 2048)
        # piece-major: [128, NB, J2, 2, 512]; d = j2*256 + r*128 + p
        xq = np.ascontiguousarray(
            x8t.reshape(J2, 2, 128, NB, 512).transpose(2, 3, 0, 1, 4))
        xsqt = (xs * xs).astype(np.float16).T          # (1024, 2048)
        xsqh = np.ascontiguousarray(
            xsqt.reshape(JJ, 128, NB, 512).transpose(1, 2, 0, 3))
        in_maps.append({
            "xq": xq,
            "xsqh": xsqh,
            "wq": wq,
            "wx": wx8,
            "wxxh": wxxh,
            "s2": s2,
            "bs": bs_np,
        })

    nc = _get_nc()
    res = run_bass_kernel_spmd(nc, in_maps, list(range(N_CORES)),
                               trace=_trace)
    _CACHE["last_results"] = res
    outT = np.concatenate([res.results[c]["outT"] for c in range(N_CORES)],
                          axis=1)                      # (128, 16384)
    return np.ascontiguousarray(outT.T).astype(np.float32)


# revision 15
# speedup vs baseline: 1.0507x; 1.0507x over previous
"""MFA per-component log-likelihood kernel for 8x TRN2 NeuronCores.

Math: out[n,k] = base[k] + sum_m (x_n . g_km)^2 + x_n . Wx_k + (x_n^2) . Wxx_k
with g/Wx/base from the Woodbury factorization (host-side, tiny).

Device strategy (per core, N_SHARD=2048 rows, output TRANSPOSED [K, n]):
  - Weights-stationary fp8 (e4m3) DoubleRow matmuls for the factor ("quad")
    columns: stationary = Gw block [128d, 2, 128cols], moving = x_fp8
    [128d, 2, 512n] -> psum y-block [128cols, 512n], 256-deep contraction
    per streamed column (2x fp32 MAC rate).
  - ScalarE squares psum with scale 1/64 into fp8 "sq" pairs.
  - Group-of-16 reduction on the PE: fp8 DoubleRow matmul against a
    constant 0/4 block-indicator matrix (S2), accumulating quad directly
    into the per-n-block accumulator psum bank [128k, 512n].
  - Linear term x.Wx (fp8 DoubleRow, same moving x) and x^2.Wxx (fp16 for
    precision -- fp8 wxx/xsq triples the output error) accumulate into the
    same bank, placed before the block's final group-sum so the PE never
    waits on the scalar engine.
  - DVE adds base (per-partition scalar) while copying acc psum -> SBUF
    fp16, DMA out per block (overlapped with the next block's matmuls).
    Host transposes [K,N] -> [N,K] and widens to fp32.

Sharding: rows N=16384 split across 8 cores; params replicated.
DMA: all tensors are stored PIECE-MAJOR in DRAM and SBUF so every piece
transfer is contiguous per partition (1-8KB runs; sub-KB strided runs
crawl at ~12GB/s on the DGE rings). Pieces ride the two hardware rings
(sync + scalar) ordered by first-use time; tiny constants and the block
outputs use the gpsimd software ring.
"""

import math

import numpy as np

K, D_FEAT, L_FAC, N = 128, 1024, 16, 16384
N_CORES = 8
N_SHARD = N // N_CORES            # 2048 rows per core
NB = N_SHARD // 512               # 4 moving blocks of 512 rows
J2 = 4                            # DoubleRow contraction chunks (256 each)
JJ = 8                            # fp16 xsq chunks (128 each)
CB = 16                           # 128-col blocks of factor columns
NPAIR = CB // 2                   # S2 pair matmuls per n-block
WPC = 4                           # wq pieces (512 factor cols each)
GCOLS = K * L_FAC                 # 2048 factor columns
SG = 32.0                         # Gw fp8 scale
SQ_SCALE = 1.0 / 64.0             # scalar: sq = (psum/64)^2 = y^2/4
S2_VAL = 4.0                      # un-scales sq in the group-sum matmul

_CACHE = {}


def _get_nc():
    if "nc" in _CACHE:
        return _CACHE["nc"]

    import concourse.bass as bass
    import concourse.tile as tile
    from concourse import bacc, mybir

    f32 = mybir.dt.float32
    f16 = mybir.dt.float16
    f8 = mybir.dt.float8e4
    DR = mybir.MatmulPerfMode.DoubleRow
    nc = bacc.Bacc("TRN2", target_bir_lowering=False, debug=False,
                   num_devices=N_CORES)

    xq = nc.dram_tensor("xq", [128, NB, J2, 2, 512], f8, kind="ExternalInput").ap()
    xsqh = nc.dram_tensor("xsqh", [128, NB, JJ, 512], f16, kind="ExternalInput").ap()
    wq = nc.dram_tensor("wq", [128, WPC, J2, 2, 512], f8, kind="ExternalInput").ap()
    wx = nc.dram_tensor("wx", [128, J2, 2, K], f8, kind="ExternalInput").ap()
    wxxh = nc.dram_tensor("wxxh", [128, JJ, K], f16, kind="ExternalInput").ap()
    s2 = nc.dram_tensor("s2", [128, NPAIR, 2, K], f8, kind="ExternalInput").ap()
    bs = nc.dram_tensor("bs", [128, 1], f32, kind="ExternalInput").ap()
    outT = nc.dram_tensor("outT", [128, N_SHARD], f16, kind="ExternalOutput").ap()

    with tile.TileContext(nc) as tc:
        with (
            tc.tile_pool(name="singles", bufs=1) as singles,
            tc.tile_pool(name="sqpool", bufs=4) as sqpool,
            tc.tile_pool(name="upool", bufs=2) as upool,
            tc.tile_pool(name="qp", bufs=6, space="PSUM") as qp,
            tc.tile_pool(name="accp", bufs=2, space="PSUM") as accp,
        ):
            wq_s = singles.tile([128, WPC, J2, 2, 512], f8, tag="wq")
            wx_s = singles.tile([128, J2, 2, K], f8, tag="wx")
            xq_s = singles.tile([128, NB, J2, 2, 512], f8, tag="xq")
            xs_s = singles.tile([128, NB, JJ, 512], f16, tag="xs")
            wxx_s = singles.tile([128, JJ, K], f16, tag="wxx")
            s2_s = singles.tile([128, NPAIR, 2, K], f8, tag="s2")
            bs_s = singles.tile([128, 1], f32, tag="bs")
            dmy = singles.tile([128, 2, 512], f8, tag="dmy")

            nc.vector.memset(dmy, 0)

            # Ring plan (aggregate early bandwidth ~150GB/s pins the stream
            # start at ~9us; schedule pieces by first-use):
            # - scalar ring: ONLY xq0+xq1, issued wait-free before any
            #   ACTIVATE (a DMA issue waiting on a recycled DGE semaphore
            #   would block the queue and stall every square behind it).
            # - sync ring: wq pieces (the stream's pacing input), wx, xs0/1.
            # - gpsimd ring: s2 first (needed ~12us), tiny constants, late
            #   x/xsq pieces, block outputs.
            nc.scalar.dma_start(out=xq_s[:, 0], in_=xq[:, 0])
            nc.scalar.dma_start(out=xq_s[:, 1], in_=xq[:, 1])
            nc.sync.dma_start(out=wq_s[:, 0], in_=wq[:, 0])
            nc.sync.dma_start(out=wq_s[:, 1], in_=wq[:, 1])
            nc.sync.dma_start(out=wq_s[:, 2], in_=wq[:, 2])
            nc.sync.dma_start(out=wq_s[:, 3], in_=wq[:, 3])
            nc.sync.dma_start(out=wx_s, in_=wx)
            nc.sync.dma_start(out=xs_s[:, 0], in_=xsqh[:, 0])
            nc.sync.dma_start(out=xs_s[:, 1], in_=xsqh[:, 1])
            nc.gpsimd.dma_start(out=s2_s, in_=s2)
            nc.gpsimd.dma_start(out=bs_s, in_=bs)
            nc.gpsimd.dma_start(out=wxx_s, in_=wxxh)
            nc.gpsimd.dma_start(out=xq_s[:, 2], in_=xq[:, 2])
            nc.gpsimd.dma_start(out=xq_s[:, 3], in_=xq[:, 3])
            nc.gpsimd.dma_start(out=xs_s[:, 2], in_=xsqh[:, 2])
            nc.gpsimd.dma_start(out=xs_s[:, 3], in_=xsqh[:, 3])

            # Dummy matmuls ramp the PE p-state while the first DMAs land.
            for _ in range(6):
                wrm = qp.tile([128, 512], f32, tag="q")
                nc.tensor.matmul(wrm, dmy[:, :, 0:128], dmy,
                                 start=True, stop=True, perf_mode=DR)

            state = {}

            def emit_pairs(nb, pairs):
                st = state.setdefault(nb, {"acc": None, "pending": None})
                if st["acc"] is None:
                    acc_t = accp.tile([128, 512], f32, tag="acc")
                    st["acc"] = acc_t
                acc = st["acc"]
                for pair in pairs:
                    sq_t = sqpool.tile([128, 2, 512], f8, tag="sq")
                    for r in range(2):
                        cb = 2 * pair + r
                        q = qp.tile([128, 512], f32, tag="q")
                        for j2 in range(J2):
                            nc.tensor.matmul(
                                q,
                                wq_s[:, cb // 4, j2, :,
                                     (cb % 4) * 128:(cb % 4 + 1) * 128],
                                xq_s[:, nb, j2],
                                start=(j2 == 0), stop=(j2 == J2 - 1),
                                perf_mode=DR)
                        nc.scalar.activation(
                            sq_t[:, r, :], q,
                            mybir.ActivationFunctionType.Square,
                            scale=SQ_SCALE)
                    # defer the group-sum one pair so the square can finish
                    if st["pending"] is not None:
                        p_pair, p_sq = st["pending"]
                        nc.tensor.matmul(acc, s2_s[:, p_pair], p_sq,
                                         start=(p_pair == 0), stop=False,
                                         perf_mode=DR)
                    st["pending"] = (pair, sq_t)

            def emit_block_tail(nb, split_out):
                st = state[nb]
                acc = st["acc"]
                nbs = slice(nb * 512, (nb + 1) * 512)
                # linear terms x.Wx and x^2.Wxx run while the scalar engine
                # finishes the last pair's squares
                for j2 in range(J2):
                    nc.tensor.matmul(acc, wx_s[:, j2], xq_s[:, nb, j2],
                                     start=False, stop=False, perf_mode=DR)
                for jj in range(JJ):
                    nc.tensor.matmul(acc, wxx_s[:, jj], xs_s[:, nb, jj],
                                     start=False, stop=False)
                p_pair, p_sq = st["pending"]
                nc.tensor.matmul(acc, s2_s[:, p_pair], p_sq,
                                 start=False, stop=True, perf_mode=DR)
                # block output immediately: overlaps later matmuls. The last
                # block splits across the two (by then idle) hw rings.
                if not split_out:
                    u = upool.tile([128, 512], f16, tag="u")
                    nc.vector.tensor_scalar_add(out=u, in0=acc, scalar1=bs_s)
                    nc.gpsimd.dma_start(out=outT[:, nbs], in_=u)
                else:
                    # partition-halved across the (by now idle) hw rings
                    u = upool.tile([128, 512], f16, tag="u")
                    nc.vector.tensor_scalar_add(out=u, in0=acc, scalar1=bs_s)
                    nc.sync.dma_start(out=outT[0:64, nbs], in_=u[0:64])
                    nc.scalar.dma_start(out=outT[64:128, nbs], in_=u[64:128])

            for nb in range(NB):
                emit_pairs(nb, range(NPAIR))
                emit_block_tail(nb, split_out=(nb == NB - 1))

    nc.finalize()
    _CACHE["nc"] = nc
    return nc


def _host_params(PI, MU, A, D):
    import ml_dtypes
    FP8 = ml_dtypes.float8_e4m3

    PI64 = PI.astype(np.float64)
    MU64 = MU.astype(np.float64)
    A64 = A.astype(np.float64)
    D64 = D.astype(np.float64)

    iD = D64 ** -2.0                                   # (K, d)
    iDA = iD[:, :, None] * A64                         # (K, d, l)
    Lm = np.eye(L_FAC)[None] + np.einsum("kdl,kdm->klm", A64, iDA)
    iL = np.linalg.inv(Lm)
    C = np.linalg.cholesky(iL)                         # iL = C C^T
    s = 1.0 / math.sqrt(2.0)
    G = np.einsum("kdl,klm->kdm", iDA, C) * s          # (K, d, l)
    b = np.einsum("kd,kdl->kl", MU64, iDA)             # (K, l)
    h = np.einsum("kl,klm->km", b, C) * s              # (K, l)

    Gw = G.transpose(1, 0, 2).reshape(D_FEAT, GCOLS)   # col k*16+m
    Wx = (iD * MU64).T - 2.0 * np.einsum("kdm,km->kd", G, h).T
    Wxx = -0.5 * iD.T                                  # (d, K)

    det_L = np.linalg.slogdet(Lm)[1]
    log_det_sigma = det_L - np.sum(np.log(iD), axis=1)
    c1 = np.sum(iD * MU64 * MU64, axis=1)
    hsq = np.sum(h * h, axis=1)
    base = PI64 - 0.5 * (D_FEAT * math.log(2.0 * math.pi)
                         + log_det_sigma + c1) + hsq

    # wq piece-major: [128, WPC, J2, 2, 512], piece pc = factor cols
    # [pc*512, (pc+1)*512); d index = j2*256 + r*128 + p.
    gw8 = (Gw * SG).astype(np.float32).astype(FP8)     # (d, 2048)
    wq = np.ascontiguousarray(
        gw8.reshape(J2, 2, 128, WPC, 512).transpose(2, 3, 0, 1, 4))
    wx8 = np.ascontiguousarray(
        Wx.astype(np.float32).astype(FP8)
        .reshape(J2, 2, 128, K).transpose(2, 0, 1, 3))

    wxxh = np.ascontiguousarray(
        Wxx.astype(np.float32).reshape(JJ, 128, K).transpose(1, 0, 2)
    ).astype(np.float16)

    s2 = np.zeros((128, NPAIR, 2, K), dtype=np.float32)
    p_idx = np.arange(128)
    for pair in range(NPAIR):
        for r in range(2):
            cb = 2 * pair + r
            s2[p_idx, pair, r, cb * 8 + p_idx // 16] = S2_VAL
    s2 = s2.astype(FP8)

    bs_np = np.ascontiguousarray(base.astype(np.float32).reshape(128, 1))
    return wq, wx8, wxxh, s2, bs_np


def kernel(x, PI, MU, A, D, _trace=False):
    from concourse.bass_utils import run_bass_kernel_spmd
    import ml_dtypes
    FP8 = ml_dtypes.float8_e4m3

    x = np.asarray(x, dtype=np.float32)
    wq, wx8, wxxh, s2, bs_np = _host_params(
        np.asarray(PI), np.asarray(MU), np.asarray(A), np.asarray(D))

    in_maps = []
    for c in range(N_CORES):
        xs = x[c * N_SHARD:(c + 1) * N_SHARD]          # (2048, 1024)
        x8t = xs.astype(FP8).T                         # (1024, 2048)
        # piece-major: [128, NB, J2, 2, 512]; d = j2*256 + r*128 + p
        xq = np.ascontiguousarray(
            x8t.reshape(J2, 2, 128, NB, 512).transpose(2, 3, 0, 1, 4))
        xsqt = (xs * xs).astype(np.float16).T          # (1024, 2048)
        xsqh = np.ascontiguousarray(
            xsqt.reshape(JJ, 128, NB, 512).transpose(1, 2, 0, 3))
        in_maps.append({
            "xq": xq,
            "xsqh": xsqh,
            "wq": wq,
            "wx": wx8,
            "wxxh": wxxh,
            "s2": s2,
            "bs": bs_np,
        })

    nc = _get_nc()
    res = run_bass_kernel_spmd(nc, in_maps, list(range(N_CORES)),
                               trace=_trace)
    _CACHE["last_results"] = res
    outT = np.concatenate([res.results[c]["outT"] for c in range(N_CORES)],
                          axis=1)                      # (128, 16384)
    return np.ascontiguousarray(outT.T).astype(np.float32)


# revision 16
# speedup vs baseline: 1.0528x; 1.0020x over previous
"""MFA per-component log-likelihood kernel for 8x TRN2 NeuronCores.

Math: out[n,k] = base[k] + sum_m (x_n . g_km)^2 + x_n . Wx_k + (x_n^2) . Wxx_k
with g/Wx/base from the Woodbury factorization (host-side, tiny).

Device strategy (per core, N_SHARD=2048 rows, output TRANSPOSED [K, n]):
  - Weights-stationary fp8 (e4m3) DoubleRow matmuls for the factor ("quad")
    columns: stationary = Gw block [128d, 2, 128cols], moving = x_fp8
    [128d, 2, 512n] -> psum y-block [128cols, 512n], 256-deep contraction
    per streamed column (2x fp32 MAC rate).
  - ScalarE squares psum with scale 1/64 into fp8 "sq" pairs.
  - Group-of-16 reduction on the PE: fp8 DoubleRow matmul against a
    constant 0/4 block-indicator matrix (S2), accumulating quad directly
    into the per-n-block accumulator psum bank [128k, 512n].
  - Linear term x.Wx (fp8 DoubleRow, same moving x) and x^2.Wxx (fp16 for
    precision -- fp8 wxx/xsq triples the output error) accumulate into the
    same bank, placed before the block's final group-sum so the PE never
    waits on the scalar engine.
  - DVE adds base (per-partition scalar) while copying acc psum -> SBUF
    fp16, DMA out per block (overlapped with the next block's matmuls).
    Host transposes [K,N] -> [N,K] and widens to fp32.

Sharding: rows N=16384 split across 8 cores; params replicated.
DMA: all tensors are stored PIECE-MAJOR in DRAM and SBUF so every piece
transfer is contiguous per partition (1-8KB runs; sub-KB strided runs
crawl at ~12GB/s on the DGE rings). Pieces ride the two hardware rings
(sync + scalar) ordered by first-use time; tiny constants and the block
outputs use the gpsimd software ring.
"""

import math

import numpy as np

K, D_FEAT, L_FAC, N = 128, 1024, 16, 16384
N_CORES = 8
N_SHARD = N // N_CORES            # 2048 rows per core
NB = N_SHARD // 512               # 4 moving blocks of 512 rows
J2 = 4                            # DoubleRow contraction chunks (256 each)
JJ = 8                            # fp16 xsq chunks (128 each)
CB = 16                           # 128-col blocks of factor columns
NPAIR = CB // 2                   # S2 pair matmuls per n-block
WPC = 4                           # wq pieces (512 factor cols each)
GCOLS = K * L_FAC                 # 2048 factor columns
SG = 32.0                         # Gw fp8 scale
SQ_SCALE = 1.0 / 64.0             # scalar: sq = (psum/64)^2 = y^2/4
S2_VAL = 4.0                      # un-scales sq in the group-sum matmul

_CACHE = {}


def _get_nc():
    if "nc" in _CACHE:
        return _CACHE["nc"]

    import concourse.bass as bass
    import concourse.tile as tile
    from concourse import bacc, mybir

    f32 = mybir.dt.float32
    f16 = mybir.dt.float16
    f8 = mybir.dt.float8e4
    DR = mybir.MatmulPerfMode.DoubleRow
    nc = bacc.Bacc("TRN2", target_bir_lowering=False, debug=False,
                   num_devices=N_CORES)

    xq = nc.dram_tensor("xq", [128, NB, J2, 2, 512], f8, kind="ExternalInput").ap()
    xsqh = nc.dram_tensor("xsqh", [128, NB, JJ, 512], f16, kind="ExternalInput").ap()
    wq = nc.dram_tensor("wq", [128, WPC, J2, 2, 512], f8, kind="ExternalInput").ap()
    wx = nc.dram_tensor("wx", [128, J2, 2, K], f8, kind="ExternalInput").ap()
    wxxh = nc.dram_tensor("wxxh", [128, JJ, K], f16, kind="ExternalInput").ap()
    s2 = nc.dram_tensor("s2", [128, NPAIR, 2, K], f8, kind="ExternalInput").ap()
    bs = nc.dram_tensor("bs", [128, 1], f32, kind="ExternalInput").ap()
    outT = nc.dram_tensor("outT", [128, N_SHARD], f16, kind="ExternalOutput").ap()

    with tile.TileContext(nc) as tc:
        with (
            tc.tile_pool(name="singles", bufs=1) as singles,
            tc.tile_pool(name="sqpool", bufs=4) as sqpool,
            tc.tile_pool(name="upool", bufs=2) as upool,
            tc.tile_pool(name="qp", bufs=6, space="PSUM") as qp,
            tc.tile_pool(name="accp", bufs=2, space="PSUM") as accp,
        ):
            wq_s = singles.tile([128, WPC, J2, 2, 512], f8, tag="wq")
            wx_s = singles.tile([128, J2, 2, K], f8, tag="wx")
            xq_s = singles.tile([128, NB, J2, 2, 512], f8, tag="xq")
            xs_s = singles.tile([128, NB, JJ, 512], f16, tag="xs")
            wxx_s = singles.tile([128, JJ, K], f16, tag="wxx")
            s2_s = singles.tile([128, NPAIR, 2, K], f8, tag="s2")
            bs_s = singles.tile([128, 1], f32, tag="bs")
            dmy = singles.tile([128, 2, 512], f8, tag="dmy")

            nc.vector.memset(dmy, 0)

            # Ring plan: the SYNC hw ring is by far the fastest (~200+GB/s
            # with >=4KB runs) -- all inputs ride it in first-use order,
            # exactly like the proven baseline schedule. The scalar queue
            # carries no input DMAs (a DMA issue waiting on a recycled DGE
            # semaphore would block the ACTIVATE squares behind it); gpsimd
            # only carries mid-stream block outputs.
            nc.sync.dma_start(out=s2_s, in_=s2)
            nc.sync.dma_start(out=wq_s[:, 0], in_=wq[:, 0])
            nc.sync.dma_start(out=xq_s[:, 0], in_=xq[:, 0])
            nc.sync.dma_start(out=wq_s[:, 1], in_=wq[:, 1])
            nc.sync.dma_start(out=wq_s[:, 2], in_=wq[:, 2])
            nc.sync.dma_start(out=wq_s[:, 3], in_=wq[:, 3])
            nc.sync.dma_start(out=xq_s[:, 1], in_=xq[:, 1])
            nc.sync.dma_start(out=wx_s, in_=wx)
            nc.sync.dma_start(out=bs_s, in_=bs)
            nc.sync.dma_start(out=wxx_s, in_=wxxh)
            nc.sync.dma_start(out=xs_s[:, 0], in_=xsqh[:, 0])
            nc.sync.dma_start(out=xq_s[:, 2], in_=xq[:, 2])
            nc.sync.dma_start(out=xs_s[:, 1], in_=xsqh[:, 1])
            nc.sync.dma_start(out=xq_s[:, 3], in_=xq[:, 3])
            nc.sync.dma_start(out=xs_s[:, 2], in_=xsqh[:, 2])
            nc.sync.dma_start(out=xs_s[:, 3], in_=xsqh[:, 3])

            # Dummy matmuls ramp the PE p-state while the first DMAs land.
            for _ in range(6):
                wrm = qp.tile([128, 512], f32, tag="q")
                nc.tensor.matmul(wrm, dmy[:, :, 0:128], dmy,
                                 start=True, stop=True, perf_mode=DR)

            state = {}

            def emit_pairs(nb, pairs):
                st = state.setdefault(nb, {"acc": None, "pending": None})
                if st["acc"] is None:
                    acc_t = accp.tile([128, 512], f32, tag="acc")
                    st["acc"] = acc_t
                acc = st["acc"]
                for pair in pairs:
                    sq_t = sqpool.tile([128, 2, 512], f8, tag="sq")
                    for r in range(2):
                        cb = 2 * pair + r
                        q = qp.tile([128, 512], f32, tag="q")
                        for j2 in range(J2):
                            nc.tensor.matmul(
                                q,
                                wq_s[:, cb // 4, j2, :,
                                     (cb % 4) * 128:(cb % 4 + 1) * 128],
                                xq_s[:, nb, j2],
                                start=(j2 == 0), stop=(j2 == J2 - 1),
                                perf_mode=DR)
                        nc.scalar.activation(
                            sq_t[:, r, :], q,
                            mybir.ActivationFunctionType.Square,
                            scale=SQ_SCALE)
                    # defer the group-sum one pair so the square can finish
                    if st["pending"] is not None:
                        p_pair, p_sq = st["pending"]
                        nc.tensor.matmul(acc, s2_s[:, p_pair], p_sq,
                                         start=(p_pair == 0), stop=False,
                                         perf_mode=DR)
                    st["pending"] = (pair, sq_t)

            def emit_block_tail(nb, split_out):
                st = state[nb]
                acc = st["acc"]
                nbs = slice(nb * 512, (nb + 1) * 512)
                # linear terms x.Wx and x^2.Wxx run while the scalar engine
                # finishes the last pair's squares
                for j2 in range(J2):
                    nc.tensor.matmul(acc, wx_s[:, j2], xq_s[:, nb, j2],
                                     start=False, stop=False, perf_mode=DR)
                for jj in range(JJ):
                    nc.tensor.matmul(acc, wxx_s[:, jj], xs_s[:, nb, jj],
                                     start=False, stop=False)
                p_pair, p_sq = st["pending"]
                nc.tensor.matmul(acc, s2_s[:, p_pair], p_sq,
                                 start=False, stop=True, perf_mode=DR)
                # block output immediately: overlaps later matmuls. The last
                # block splits across the two (by then idle) hw rings.
                if not split_out:
                    u = upool.tile([128, 512], f16, tag="u")
                    nc.vector.tensor_scalar_add(out=u, in0=acc, scalar1=bs_s)
                    nc.gpsimd.dma_start(out=outT[:, nbs], in_=u)
                else:
                    # partition-halved across the (by now idle) hw rings
                    u = upool.tile([128, 512], f16, tag="u")
                    nc.vector.tensor_scalar_add(out=u, in0=acc, scalar1=bs_s)
                    nc.sync.dma_start(out=outT[0:64, nbs], in_=u[0:64])
                    nc.scalar.dma_start(out=outT[64:128, nbs], in_=u[64:128])

            for nb in range(NB):
                emit_pairs(nb, range(NPAIR))
                emit_block_tail(nb, split_out=(nb == NB - 1))

    nc.finalize()
    _CACHE["nc"] = nc
    return nc


def _host_params(PI, MU, A, D):
    import ml_dtypes
    FP8 = ml_dtypes.float8_e4m3

    PI64 = PI.astype(np.float64)
    MU64 = MU.astype(np.float64)
    A64 = A.astype(np.float64)
    D64 = D.astype(np.float64)

    iD = D64 ** -2.0                                   # (K, d)
    iDA = iD[:, :, None] * A64                         # (K, d, l)
    Lm = np.eye(L_FAC)[None] + np.einsum("kdl,kdm->klm", A64, iDA)
    iL = np.linalg.inv(Lm)
    C = np.linalg.cholesky(iL)                         # iL = C C^T
    s = 1.0 / math.sqrt(2.0)
    G = np.einsum("kdl,klm->kdm", iDA, C) * s          # (K, d, l)
    b = np.einsum("kd,kdl->kl", MU64, iDA)             # (K, l)
    h = np.einsum("kl,klm->km", b, C) * s              # (K, l)

    Gw = G.transpose(1, 0, 2).reshape(D_FEAT, GCOLS)   # col k*16+m
    Wx = (iD * MU64).T - 2.0 * np.einsum("kdm,km->kd", G, h).T
    Wxx = -0.5 * iD.T                                  # (d, K)

    det_L = np.linalg.slogdet(Lm)[1]
    log_det_sigma = det_L - np.sum(np.log(iD), axis=1)
    c1 = np.sum(iD * MU64 * MU64, axis=1)
    hsq = np.sum(h * h, axis=1)
    base = PI64 - 0.5 * (D_FEAT * math.log(2.0 * math.pi)
                         + log_det_sigma + c1) + hsq

    # wq piece-major: [128, WPC, J2, 2, 512], piece pc = factor cols
    # [pc*512, (pc+1)*512); d index = j2*256 + r*128 + p.
    gw8 = (Gw * SG).astype(np.float32).astype(FP8)     # (d, 2048)
    wq = np.ascontiguousarray(
        gw8.reshape(J2, 2, 128, WPC, 512).transpose(2, 3, 0, 1, 4))
    wx8 = np.ascontiguousarray(
        Wx.astype(np.float32).astype(FP8)
        .reshape(J2, 2, 128, K).transpose(2, 0, 1, 3))

    wxxh = np.ascontiguousarray(
        Wxx.astype(np.float32).reshape(JJ, 128, K).transpose(1, 0, 2)
    ).astype(np.float16)

    s2 = np.zeros((128, NPAIR, 2, K), dtype=np.float32)
    p_idx = np.arange(128)
    for pair in range(NPAIR):
        for r in range(2):
            cb = 2 * pair + r
            s2[p_idx, pair, r, cb * 8 + p_idx // 16] = S2_VAL
    s2 = s2.astype(FP8)

    bs_np = np.ascontiguousarray(base.astype(np.float32).reshape(128, 1))
    return wq, wx8, wxxh, s2, bs_np


def kernel(x, PI, MU, A, D, _trace=False):
    from concourse.bass_utils import run_bass_kernel_spmd
    import ml_dtypes
    FP8 = ml_dtypes.float8_e4m3

    x = np.asarray(x, dtype=np.float32)
    wq, wx8, wxxh, s2, bs_np = _host_params(
        np.asarray(PI), np.asarray(MU), np.asarray(A), np.asarray(D))

    in_maps = []
    for c in range(N_CORES):
        xs = x[c * N_SHARD:(c + 1) * N_SHARD]          # (2048, 1024)
        x8t = xs.astype(FP8).T                         # (1024, 2048)
        # piece-major: [128, NB, J2, 2, 512]; d = j2*256 + r*128 + p
        xq = np.ascontiguousarray(
            x8t.reshape(J2, 2, 128, NB, 512).transpose(2, 3, 0, 1, 4))
        xsqt = (xs * xs).astype(np.float16).T          # (1024, 2048)
        xsqh = np.ascontiguousarray(
            xsqt.reshape(JJ, 128, NB, 512).transpose(1, 2, 0, 3))
        in_maps.append({
            "xq": xq,
            "xsqh": xsqh,
            "wq": wq,
            "wx": wx8,
            "wxxh": wxxh,
            "s2": s2,
            "bs": bs_np,
        })

    nc = _get_nc()
    res = run_bass_kernel_spmd(nc, in_maps, list(range(N_CORES)),
                               trace=_trace)
    _CACHE["last_results"] = res
    outT = np.concatenate([res.results[c]["outT"] for c in range(N_CORES)],
                          axis=1)                      # (128, 16384)
    return np.ascontiguousarray(outT.T).astype(np.float32)


# revision 17
# speedup vs baseline: 1.0841x; 1.0297x over previous
"""MFA per-component log-likelihood kernel for 8x TRN2 NeuronCores.

Math: out[n,k] = base[k] + sum_m (x_n . g_km)^2 + x_n . Wx_k + (x_n^2) . Wxx_k
with g/Wx/base from the Woodbury factorization (host-side, tiny).

Device strategy (per core, N_SHARD=2048 rows, output TRANSPOSED [K, n]):
  - Weights-stationary fp8 (e4m3) DoubleRow matmuls for the factor ("quad")
    columns: stationary = Gw block [128d, 2, 128cols], moving = x_fp8
    [128d, 2, 512n] -> psum y-block [128cols, 512n], 256-deep contraction
    per streamed column (2x fp32 MAC rate).
  - ScalarE squares psum with scale 1/64 into fp8 "sq" pairs.
  - Group-of-16 reduction on the PE: fp8 DoubleRow matmul against a
    constant 0/4 block-indicator matrix (S2), accumulating quad directly
    into the per-n-block accumulator psum bank [128k, 512n].
  - Linear term x.Wx (fp8 DoubleRow, same moving x) and x^2.Wxx (fp16 for
    precision -- fp8 wxx/xsq triples the output error) accumulate into the
    same bank, placed before the block's final group-sum so the PE never
    waits on the scalar engine.
  - DVE adds base (per-partition scalar) while copying acc psum -> SBUF
    fp16, DMA out per block (overlapped with the next block's matmuls).
    Host transposes [K,N] -> [N,K] and widens to fp32.

Sharding: rows N=16384 split across 8 cores; params replicated.
DMA: all tensors are stored PIECE-MAJOR in DRAM and SBUF so every piece
transfer is contiguous per partition (1-8KB runs; sub-KB strided runs
crawl at ~12GB/s on the DGE rings). Pieces ride the two hardware rings
(sync + scalar) ordered by first-use time; tiny constants and the block
outputs use the gpsimd software ring.
"""

import math

import numpy as np

K, D_FEAT, L_FAC, N = 128, 1024, 16, 16384
N_CORES = 8
N_SHARD = N // N_CORES            # 2048 rows per core
NB = N_SHARD // 512               # 4 moving blocks of 512 rows
J2 = 4                            # DoubleRow contraction chunks (256 each)
JJ = 8                            # fp16 xsq chunks (128 each)
CB = 16                           # 128-col blocks of factor columns
NPAIR = CB // 2                   # S2 pair matmuls per n-block
WPC = 4                           # wq pieces (512 factor cols each)
GCOLS = K * L_FAC                 # 2048 factor columns
SG = 32.0                         # Gw fp8 scale
SQ_SCALE = 1.0 / 64.0             # scalar: sq = (psum/64)^2 = y^2/4
S2_VAL = 4.0                      # un-scales sq in the group-sum matmul

_CACHE = {}


def _get_nc():
    if "nc" in _CACHE:
        return _CACHE["nc"]

    import concourse.bass as bass
    import concourse.tile as tile
    from concourse import bacc, mybir

    f32 = mybir.dt.float32
    f16 = mybir.dt.float16
    f8 = mybir.dt.float8e4
    DR = mybir.MatmulPerfMode.DoubleRow
    nc = bacc.Bacc("TRN2", target_bir_lowering=False, debug=False,
                   num_devices=N_CORES)

    xq = nc.dram_tensor("xq", [128, NB, J2, 2, 512], f8, kind="ExternalInput").ap()
    xsqh = nc.dram_tensor("xsqh", [128, NB, JJ, 512], f16, kind="ExternalInput").ap()
    wq = nc.dram_tensor("wq", [128, WPC, J2, 2, 512], f8, kind="ExternalInput").ap()
    wx = nc.dram_tensor("wx", [128, J2, 2, K], f8, kind="ExternalInput").ap()
    wxxh = nc.dram_tensor("wxxh", [128, JJ, K], f16, kind="ExternalInput").ap()
    s2 = nc.dram_tensor("s2", [128, NPAIR, 2, K], f8, kind="ExternalInput").ap()
    bs = nc.dram_tensor("bs", [128, 1], f32, kind="ExternalInput").ap()
    outT = nc.dram_tensor("outT", [128, N_SHARD], f16, kind="ExternalOutput").ap()

    with tile.TileContext(nc) as tc:
        with (
            tc.tile_pool(name="singles", bufs=1) as singles,
            tc.tile_pool(name="sqpool", bufs=4) as sqpool,
            tc.tile_pool(name="upool", bufs=2) as upool,
            tc.tile_pool(name="qp", bufs=6, space="PSUM") as qp,
            tc.tile_pool(name="accp", bufs=2, space="PSUM") as accp,
        ):
            wq_s = singles.tile([128, WPC, J2, 2, 512], f8, tag="wq")
            wx_s = singles.tile([128, J2, 2, K], f8, tag="wx")
            xq_s = singles.tile([128, NB, J2, 2, 512], f8, tag="xq")
            xs_s = singles.tile([128, NB, JJ, 512], f16, tag="xs")
            wxx_s = singles.tile([128, JJ, K], f16, tag="wxx")
            s2_s = singles.tile([128, NPAIR, 2, K], f8, tag="s2")
            bs_s = singles.tile([128, 1], f32, tag="bs")
            dmy = singles.tile([128, 2, 512], f8, tag="dmy")

            nc.vector.memset(dmy, 0)

            # Ring plan: the SYNC hw ring is by far the fastest (~200+GB/s
            # with >=4KB runs) -- all inputs ride it in first-use order,
            # exactly like the proven baseline schedule. The scalar queue
            # carries no input DMAs (a DMA issue waiting on a recycled DGE
            # semaphore would block the ACTIVATE squares behind it); gpsimd
            # only carries mid-stream block outputs.
            nc.sync.dma_start(out=wq_s[:, 0], in_=wq[:, 0])
            nc.sync.dma_start(out=xq_s[:, 0], in_=xq[:, 0])
            nc.sync.dma_start(out=s2_s, in_=s2)
            nc.sync.dma_start(out=wq_s[:, 1], in_=wq[:, 1])
            nc.sync.dma_start(out=wq_s[:, 2], in_=wq[:, 2])
            nc.sync.dma_start(out=xq_s[:, 1], in_=xq[:, 1])
            nc.sync.dma_start(out=wq_s[:, 3], in_=wq[:, 3])
            nc.sync.dma_start(out=wx_s, in_=wx)
            nc.sync.dma_start(out=bs_s, in_=bs)
            nc.sync.dma_start(out=wxx_s, in_=wxxh)
            nc.sync.dma_start(out=xs_s[:, 0], in_=xsqh[:, 0])
            nc.sync.dma_start(out=xq_s[:, 2], in_=xq[:, 2])
            nc.sync.dma_start(out=xs_s[:, 1], in_=xsqh[:, 1])
            nc.sync.dma_start(out=xq_s[:, 3], in_=xq[:, 3])
            nc.sync.dma_start(out=xs_s[:, 2], in_=xsqh[:, 2])
            nc.sync.dma_start(out=xs_s[:, 3], in_=xsqh[:, 3])

            # Dummy matmuls bridge the PE continuously until the first real
            # data lands: an idle gap would reset the p-state ramp and the
            # first ~14 real matmuls would run at half clock (~4us lost).
            for _ in range(12):
                wrm = qp.tile([128, 512], f32, tag="q")
                nc.tensor.matmul(wrm, dmy[:, :, 0:128], dmy,
                                 start=True, stop=True, perf_mode=DR)

            state = {}

            def emit_pairs(nb, pairs):
                st = state.setdefault(nb, {"acc": None, "pending": None})
                if st["acc"] is None:
                    acc_t = accp.tile([128, 512], f32, tag="acc")
                    st["acc"] = acc_t
                acc = st["acc"]
                for pair in pairs:
                    sq_t = sqpool.tile([128, 2, 512], f8, tag="sq")
                    for r in range(2):
                        cb = 2 * pair + r
                        q = qp.tile([128, 512], f32, tag="q")
                        for j2 in range(J2):
                            nc.tensor.matmul(
                                q,
                                wq_s[:, cb // 4, j2, :,
                                     (cb % 4) * 128:(cb % 4 + 1) * 128],
                                xq_s[:, nb, j2],
                                start=(j2 == 0), stop=(j2 == J2 - 1),
                                perf_mode=DR)
                        nc.scalar.activation(
                            sq_t[:, r, :], q,
                            mybir.ActivationFunctionType.Square,
                            scale=SQ_SCALE)
                    # defer the group-sum one pair so the square can finish
                    if st["pending"] is not None:
                        p_pair, p_sq = st["pending"]
                        nc.tensor.matmul(acc, s2_s[:, p_pair], p_sq,
                                         start=(p_pair == 0), stop=False,
                                         perf_mode=DR)
                    st["pending"] = (pair, sq_t)

            def emit_block_tail(nb, split_out):
                st = state[nb]
                acc = st["acc"]
                nbs = slice(nb * 512, (nb + 1) * 512)
                # linear terms x.Wx and x^2.Wxx run while the scalar engine
                # finishes the last pair's squares
                for j2 in range(J2):
                    nc.tensor.matmul(acc, wx_s[:, j2], xq_s[:, nb, j2],
                                     start=False, stop=False, perf_mode=DR)
                for jj in range(JJ):
                    nc.tensor.matmul(acc, wxx_s[:, jj], xs_s[:, nb, jj],
                                     start=False, stop=False)
                p_pair, p_sq = st["pending"]
                nc.tensor.matmul(acc, s2_s[:, p_pair], p_sq,
                                 start=False, stop=True, perf_mode=DR)
                # block output immediately: overlaps later matmuls. The last
                # block splits across the two (by then idle) hw rings.
                if not split_out:
                    u = upool.tile([128, 512], f16, tag="u")
                    nc.vector.tensor_scalar_add(out=u, in0=acc, scalar1=bs_s)
                    nc.gpsimd.dma_start(out=outT[:, nbs], in_=u)
                else:
                    # partition-halved across the (by now idle) hw rings
                    u = upool.tile([128, 512], f16, tag="u")
                    nc.vector.tensor_scalar_add(out=u, in0=acc, scalar1=bs_s)
                    nc.sync.dma_start(out=outT[0:64, nbs], in_=u[0:64])
                    nc.scalar.dma_start(out=outT[64:128, nbs], in_=u[64:128])

            for nb in range(NB):
                emit_pairs(nb, range(NPAIR))
                emit_block_tail(nb, split_out=(nb == NB - 1))

    nc.finalize()
    _CACHE["nc"] = nc
    return nc


def _host_params(PI, MU, A, D):
    import ml_dtypes
    FP8 = ml_dtypes.float8_e4m3

    PI64 = PI.astype(np.float64)
    MU64 = MU.astype(np.float64)
    A64 = A.astype(np.float64)
    D64 = D.astype(np.float64)

    iD = D64 ** -2.0                                   # (K, d)
    iDA = iD[:, :, None] * A64                         # (K, d, l)
    Lm = np.eye(L_FAC)[None] + np.einsum("kdl,kdm->klm", A64, iDA)
    iL = np.linalg.inv(Lm)
    C = np.linalg.cholesky(iL)                         # iL = C C^T
    s = 1.0 / math.sqrt(2.0)
    G = np.einsum("kdl,klm->kdm", iDA, C) * s          # (K, d, l)
    b = np.einsum("kd,kdl->kl", MU64, iDA)             # (K, l)
    h = np.einsum("kl,klm->km", b, C) * s              # (K, l)

    Gw = G.transpose(1, 0, 2).reshape(D_FEAT, GCOLS)   # col k*16+m
    Wx = (iD * MU64).T - 2.0 * np.einsum("kdm,km->kd", G, h).T
    Wxx = -0.5 * iD.T                                  # (d, K)

    det_L = np.linalg.slogdet(Lm)[1]
    log_det_sigma = det_L - np.sum(np.log(iD), axis=1)
    c1 = np.sum(iD * MU64 * MU64, axis=1)
    hsq = np.sum(h * h, axis=1)
    base = PI64 - 0.5 * (D_FEAT * math.log(2.0 * math.pi)
                         + log_det_sigma + c1) + hsq

    # wq piece-major: [128, WPC, J2, 2, 512], piece pc = factor cols
    # [pc*512, (pc+1)*512); d index = j2*256 + r*128 + p.
    gw8 = (Gw * SG).astype(np.float32).astype(FP8)     # (d, 2048)
    wq = np.ascontiguousarray(
        gw8.reshape(J2, 2, 128, WPC, 512).transpose(2, 3, 0, 1, 4))
    wx8 = np.ascontiguousarray(
        Wx.astype(np.float32).astype(FP8)
        .reshape(J2, 2, 128, K).transpose(2, 0, 1, 3))

    wxxh = np.ascontiguousarray(
        Wxx.astype(np.float32).reshape(JJ, 128, K).transpose(1, 0, 2)
    ).astype(np.float16)

    s2 = np.zeros((128, NPAIR, 2, K), dtype=np.float32)
    p_idx = np.arange(128)
    for pair in range(NPAIR):
        for r in range(2):
            cb = 2 * pair + r
            s2[p_idx, pair, r, cb * 8 + p_idx // 16] = S2_VAL
    s2 = s2.astype(FP8)

    bs_np = np.ascontiguousarray(base.astype(np.float32).reshape(128, 1))
    return wq, wx8, wxxh, s2, bs_np


def kernel(x, PI, MU, A, D, _trace=False):
    from concourse.bass_utils import run_bass_kernel_spmd
    import ml_dtypes
    FP8 = ml_dtypes.float8_e4m3

    x = np.asarray(x, dtype=np.float32)
    wq, wx8, wxxh, s2, bs_np = _host_params(
        np.asarray(PI), np.asarray(MU), np.asarray(A), np.asarray(D))

    in_maps = []
    for c in range(N_CORES):
        xs = x[c * N_SHARD:(c + 1) * N_SHARD]          # (2048, 1024)
        x8t = xs.astype(FP8).T                         # (1024, 2048)
        # piece-major: [128, NB, J2, 2, 512]; d = j2*256 + r*128 + p
        xq = np.ascontiguousarray(
            x8t.reshape(J2, 2, 128, NB, 512).transpose(2, 3, 0, 1, 4))
        xsqt = (xs * xs).astype(np.float16).T          # (1024, 2048)
        xsqh = np.ascontiguousarray(
            xsqt.reshape(JJ, 128, NB, 512).transpose(1, 2, 0, 3))
        in_maps.append({
            "xq": xq,
            "xsqh": xsqh,
            "wq": wq,
            "wx": wx8,
            "wxxh": wxxh,
            "s2": s2,
            "bs": bs_np,
        })

    nc = _get_nc()
    res = run_bass_kernel_spmd(nc, in_maps, list(range(N_CORES)),
                               trace=_trace)
    _CACHE["last_results"] = res
    outT = np.concatenate([res.results[c]["outT"] for c in range(N_CORES)],
                          axis=1)                      # (128, 16384)
    return np.ascontiguousarray(outT.T).astype(np.float32)


# revision 21
# speedup vs baseline: 1.0853x; 1.0011x over previous
"""MFA per-component log-likelihood kernel for 8x TRN2 NeuronCores.

Math: out[n,k] = base[k] + sum_m (x_n . g_km)^2 + x_n . Wx_k + (x_n^2) . Wxx_k
with g/Wx/base from the Woodbury factorization (host-side, tiny).

Device strategy (per core, N_SHARD=2048 rows, output TRANSPOSED [K, n]):
  - Weights-stationary fp8 (e4m3) DoubleRow matmuls for the factor ("quad")
    columns: stationary = Gw block [128d, 2, 128cols], moving = x_fp8
    [128d, 2, 512n] -> psum y-block [128cols, 512n], 256-deep contraction
    per streamed column (2x fp32 MAC rate).
  - ScalarE squares psum with scale 1/64 into fp8 "sq" pairs.
  - Group-of-16 reduction on the PE: fp8 DoubleRow matmul against a
    constant 0/4 block-indicator matrix (S2), accumulating quad directly
    into the per-n-block accumulator psum bank [128k, 512n].
  - Linear term x.Wx (fp8 DoubleRow, same moving x) and x^2.Wxx (fp16 for
    precision -- fp8 wxx/xsq triples the output error) accumulate into the
    same bank, placed before the block's final group-sum so the PE never
    waits on the scalar engine.
  - DVE adds base (per-partition scalar) while copying acc psum -> SBUF
    fp16, DMA out per block (overlapped with the next block's matmuls).
    Host transposes [K,N] -> [N,K] and widens to fp32.

Sharding: rows N=16384 split across 8 cores; params replicated.
DMA: all tensors are stored PIECE-MAJOR in DRAM and SBUF so every piece
transfer is contiguous per partition (1-8KB runs; sub-KB strided runs
crawl at ~12GB/s on the DGE rings). Pieces ride the two hardware rings
(sync + scalar) ordered by first-use time; tiny constants and the block
outputs use the gpsimd software ring.
"""

import math

import numpy as np

K, D_FEAT, L_FAC, N = 128, 1024, 16, 16384
N_CORES = 8
N_SHARD = N // N_CORES            # 2048 rows per core
NB = N_SHARD // 512               # 4 moving blocks of 512 rows
J2 = 4                            # DoubleRow contraction chunks (256 each)
JJ = 8                            # fp16 xsq chunks (128 each)
CB = 16                           # 128-col blocks of factor columns
NPAIR = CB // 2                   # S2 pair matmuls per n-block
WPC = 4                           # wq pieces (512 factor cols each)
GCOLS = K * L_FAC                 # 2048 factor columns
SG = 32.0                         # Gw fp8 scale
SQ_SCALE = 1.0 / 64.0             # scalar: sq = (psum/64)^2 = y^2/4
S2_VAL = 4.0                      # un-scales sq in the group-sum matmul

_CACHE = {}


def _get_nc():
    if "nc" in _CACHE:
        return _CACHE["nc"]

    import concourse.bass as bass
    import concourse.tile as tile
    from concourse import bacc, mybir

    f32 = mybir.dt.float32
    f16 = mybir.dt.float16
    f8 = mybir.dt.float8e4
    DR = mybir.MatmulPerfMode.DoubleRow
    nc = bacc.Bacc("TRN2", target_bir_lowering=False, debug=False,
                   num_devices=N_CORES)

    xq = nc.dram_tensor("xq", [128, NB, J2, 2, 512], f8, kind="ExternalInput").ap()
    xsqh = nc.dram_tensor("xsqh", [128, NB, JJ, 512], f16, kind="ExternalInput").ap()
    wq = nc.dram_tensor("wq", [128, WPC, J2, 2, 512], f8, kind="ExternalInput").ap()
    wx = nc.dram_tensor("wx", [128, J2, 2, K], f8, kind="ExternalInput").ap()
    wxxh = nc.dram_tensor("wxxh", [128, JJ, K], f16, kind="ExternalInput").ap()
    s2 = nc.dram_tensor("s2", [128, NPAIR, 2, K], f8, kind="ExternalInput").ap()
    bs = nc.dram_tensor("bs", [128, 1], f32, kind="ExternalInput").ap()
    outT = nc.dram_tensor("outT", [128, N_SHARD], f16, kind="ExternalOutput").ap()

    with tile.TileContext(nc) as tc:
        with (
            tc.tile_pool(name="singles", bufs=1) as singles,
            tc.tile_pool(name="sqpool", bufs=4) as sqpool,
            tc.tile_pool(name="upool", bufs=2) as upool,
            tc.tile_pool(name="qp", bufs=6, space="PSUM") as qp,
            tc.tile_pool(name="accp", bufs=2, space="PSUM") as accp,
        ):
            wq_s = singles.tile([128, WPC, J2, 2, 512], f8, tag="wq")
            wx_s = singles.tile([128, J2, 2, K], f8, tag="wx")
            xq_s = singles.tile([128, NB, J2, 2, 512], f8, tag="xq")
            xs_s = singles.tile([128, NB, JJ, 512], f16, tag="xs")
            wxx_s = singles.tile([128, JJ, K], f16, tag="wxx")
            s2_s = singles.tile([128, NPAIR, 2, K], f8, tag="s2")
            bs_s = singles.tile([128, 1], f32, tag="bs")
            dmy = singles.tile([128, 2, 512], f8, tag="dmy")

            # gpsimd finishes its framework memsets ~0.2us in and carries no
            # input DMAs, so dmy is ready ~1.3us in (vector would be ~2.9).
            nc.gpsimd.memset(dmy, 0)

            # Ring plan: the SYNC hw ring is by far the fastest (~200+GB/s
            # with >=4KB runs) -- all inputs ride it in first-use order,
            # exactly like the proven baseline schedule. The scalar queue
            # carries no input DMAs (a DMA issue waiting on a recycled DGE
            # semaphore would block the ACTIVATE squares behind it); gpsimd
            # only carries mid-stream block outputs.
            nc.sync.dma_start(out=wq_s[:, 0], in_=wq[:, 0])
            nc.sync.dma_start(out=xq_s[:, 0], in_=xq[:, 0])
            nc.sync.dma_start(out=s2_s, in_=s2)
            nc.sync.dma_start(out=wq_s[:, 1], in_=wq[:, 1])
            nc.sync.dma_start(out=wq_s[:, 2], in_=wq[:, 2])
            nc.sync.dma_start(out=xq_s[:, 1], in_=xq[:, 1])
            nc.sync.dma_start(out=wq_s[:, 3], in_=wq[:, 3])
            nc.sync.dma_start(out=wx_s, in_=wx)
            nc.sync.dma_start(out=bs_s, in_=bs)
            nc.sync.dma_start(out=wxx_s, in_=wxxh)
            nc.sync.dma_start(out=xs_s[:, 0], in_=xsqh[:, 0])
            nc.sync.dma_start(out=xq_s[:, 2], in_=xq[:, 2])
            nc.sync.dma_start(out=xs_s[:, 1], in_=xsqh[:, 1])
            nc.sync.dma_start(out=xq_s[:, 3], in_=xq[:, 3])
            nc.sync.dma_start(out=xs_s[:, 2], in_=xsqh[:, 2])
            nc.sync.dma_start(out=xs_s[:, 3], in_=xsqh[:, 3])

            # Dummy matmuls bridge the PE continuously until the first real
            # data lands: an idle gap would reset the p-state ramp and the
            # first ~14 real matmuls would run at half clock (~4us lost).
            for _ in range(12):
                wrm = qp.tile([128, 512], f32, tag="q")
                nc.tensor.matmul(wrm, dmy[:, :, 0:128], dmy,
                                 start=True, stop=True, perf_mode=DR)

            state = {}

            def emit_pairs(nb, pairs):
                st = state.setdefault(nb, {"acc": None, "pending": []})
                if st["acc"] is None:
                    acc_t = accp.tile([128, 512], f32, tag="acc")
                    st["acc"] = acc_t
                acc = st["acc"]
                for pair in pairs:
                    sq_t = sqpool.tile([128, 2, 512], f8, tag="sq")
                    for r in range(2):
                        cb = 2 * pair + r
                        q = qp.tile([128, 512], f32, tag="q")
                        for j2 in range(J2):
                            nc.tensor.matmul(
                                q,
                                wq_s[:, cb // 4, j2, :,
                                     (cb % 4) * 128:(cb % 4 + 1) * 128],
                                xq_s[:, nb, j2],
                                start=(j2 == 0), stop=(j2 == J2 - 1),
                                perf_mode=DR)
                        nc.scalar.activation(
                            sq_t[:, r, :], q,
                            mybir.ActivationFunctionType.Square,
                            scale=SQ_SCALE)
                    # defer each group-sum TWO pairs so a late square never
                    # stalls the PE
                    st["pending"].append((pair, sq_t))
                    if len(st["pending"]) > 2:
                        p_pair, p_sq = st["pending"].pop(0)
                        nc.tensor.matmul(acc, s2_s[:, p_pair], p_sq,
                                         start=(p_pair == 0), stop=False,
                                         perf_mode=DR)

            def emit_block_tail(nb, split_out):
                st = state[nb]
                acc = st["acc"]
                nbs = slice(nb * 512, (nb + 1) * 512)
                # linear terms x.Wx and x^2.Wxx run while the scalar engine
                # finishes the last pair's squares
                for j2 in range(J2):
                    nc.tensor.matmul(acc, wx_s[:, j2], xq_s[:, nb, j2],
                                     start=False, stop=False, perf_mode=DR)
                for jj in range(JJ):
                    nc.tensor.matmul(acc, wxx_s[:, jj], xs_s[:, nb, jj],
                                     start=False, stop=False)
                pend = st["pending"]
                for k, (p_pair, p_sq) in enumerate(pend):
                    nc.tensor.matmul(acc, s2_s[:, p_pair], p_sq,
                                     start=False, stop=(k == len(pend) - 1),
                                     perf_mode=DR)
                pend.clear()
                # block output immediately: overlaps later matmuls. The last
                # block goes out in column halves on the (by now idle)
                # scalar ring so the transfer overlaps the second DVE add.
                if not split_out:
                    u = upool.tile([128, 512], f16, tag="u")
                    nc.vector.tensor_scalar_add(out=u, in0=acc, scalar1=bs_s)
                    nc.gpsimd.dma_start(out=outT[:, nbs], in_=u)
                else:
                    u = upool.tile([128, 512], f16, tag="u")
                    nc.vector.tensor_scalar_add(out=u[:, 0:256],
                                                in0=acc[:, 0:256],
                                                scalar1=bs_s)
                    nc.scalar.dma_start(
                        out=outT[:, nb * 512:nb * 512 + 256], in_=u[:, 0:256])
                    nc.vector.tensor_scalar_add(out=u[:, 256:512],
                                                in0=acc[:, 256:512],
                                                scalar1=bs_s)
                    nc.scalar.dma_start(
                        out=outT[:, nb * 512 + 256:(nb + 1) * 512],
                        in_=u[:, 256:512])

            for nb in range(NB):
                emit_pairs(nb, range(NPAIR))
                emit_block_tail(nb, split_out=(nb == NB - 1))

    nc.finalize()
    _CACHE["nc"] = nc
    return nc


def _host_params(PI, MU, A, D):
    import ml_dtypes
    FP8 = ml_dtypes.float8_e4m3

    PI64 = PI.astype(np.float64)
    MU64 = MU.astype(np.float64)
    A64 = A.astype(np.float64)
    D64 = D.astype(np.float64)

    iD = D64 ** -2.0                                   # (K, d)
    iDA = iD[:, :, None] * A64                         # (K, d, l)
    Lm = np.eye(L_FAC)[None] + np.einsum("kdl,kdm->klm", A64, iDA)
    iL = np.linalg.inv(Lm)
    C = np.linalg.cholesky(iL)                         # iL = C C^T
    s = 1.0 / math.sqrt(2.0)
    G = np.einsum("kdl,klm->kdm", iDA, C) * s          # (K, d, l)
    b = np.einsum("kd,kdl->kl", MU64, iDA)             # (K, l)
    h = np.einsum("kl,klm->km", b, C) * s              # (K, l)

    Gw = G.transpose(1, 0, 2).reshape(D_FEAT, GCOLS)   # col k*16+m
    Wx = (iD * MU64).T - 2.0 * np.einsum("kdm,km->kd", G, h).T
    Wxx = -0.5 * iD.T                                  # (d, K)

    det_L = np.linalg.slogdet(Lm)[1]
    log_det_sigma = det_L - np.sum(np.log(iD), axis=1)
    c1 = np.sum(iD * MU64 * MU64, axis=1)
    hsq = np.sum(h * h, axis=1)
    base = PI64 - 0.5 * (D_FEAT * math.log(2.0 * math.pi)
                         + log_det_sigma + c1) + hsq

    # wq piece-major: [128, WPC, J2, 2, 512], piece pc = factor cols
    # [pc*512, (pc+1)*512); d index = j2*256 + r*128 + p.
    gw8 = (Gw * SG).astype(np.float32).astype(FP8)     # (d, 2048)
    wq = np.ascontiguousarray(
        gw8.reshape(J2, 2, 128, WPC, 512).transpose(2, 3, 0, 1, 4))
    wx8 = np.ascontiguousarray(
        Wx.astype(np.float32).astype(FP8)
        .reshape(J2, 2, 128, K).transpose(2, 0, 1, 3))

    wxxh = np.ascontiguousarray(
        Wxx.astype(np.float32).reshape(JJ, 128, K).transpose(1, 0, 2)
    ).astype(np.float16)

    s2 = np.zeros((128, NPAIR, 2, K), dtype=np.float32)
    p_idx = np.arange(128)
    for pair in range(NPAIR):
        for r in range(2):
            cb = 2 * pair + r
            s2[p_idx, pair, r, cb * 8 + p_idx // 16] = S2_VAL
    s2 = s2.astype(FP8)

    bs_np = np.ascontiguousarray(base.astype(np.float32).reshape(128, 1))
    return wq, wx8, wxxh, s2, bs_np


def kernel(x, PI, MU, A, D, _trace=False):
    from concourse.bass_utils import run_bass_kernel_spmd
    import ml_dtypes
    FP8 = ml_dtypes.float8_e4m3

    x = np.asarray(x, dtype=np.float32)
    wq, wx8, wxxh, s2, bs_np = _host_params(
        np.asarray(PI), np.asarray(MU), np.asarray(A), np.asarray(D))

    in_maps = []
    for c in range(N_CORES):
        xs = x[c * N_SHARD:(c + 1) * N_SHARD]          # (2048, 1024)
        x8t = xs.astype(FP8).T                         # (1024, 2048)
        # piece-major: [128, NB, J2, 2, 512]; d = j2*256 + r*128 + p
        xq = np.ascontiguousarray(
            x8t.reshape(J2, 2, 128, NB, 512).transpose(2, 3, 0, 1, 4))
        xsqt = (xs * xs).astype(np.float16).T          # (1024, 2048)
        xsqh = np.ascontiguousarray(
            xsqt.reshape(JJ, 128, NB, 512).transpose(1, 2, 0, 3))
        in_maps.append({
            "xq": xq,
            "xsqh": xsqh,
            "wq": wq,
            "wx": wx8,
            "wxxh": wxxh,
            "s2": s2,
            "bs": bs_np,
        })

    nc = _get_nc()
    res = run_bass_kernel_spmd(nc, in_maps, list(range(N_CORES)),
                               trace=_trace)
    _CACHE["last_results"] = res
    outT = np.concatenate([res.results[c]["outT"] for c in range(N_CORES)],
                          axis=1)                      # (128, 16384)
    return np.ascontiguousarray(outT.T).astype(np.float32)


# revision 24
# speedup vs baseline: 1.0923x; 1.0065x over previous
"""MFA per-component log-likelihood kernel for 8x TRN2 NeuronCores.

Math: out[n,k] = base[k] + sum_m (x_n . g_km)^2 + x_n . Wx_k + (x_n^2) . Wxx_k
with g/Wx/base from the Woodbury factorization (host-side, tiny).

Device strategy (per core, N_SHARD=2048 rows, output TRANSPOSED [K, n]):
  - Weights-stationary fp8 (e4m3) DoubleRow matmuls for the factor ("quad")
    columns: stationary = Gw block [128d, 2, 128cols], moving = x_fp8
    [128d, 2, 512n] -> psum y-block [128cols, 512n], 256-deep contraction
    per streamed column (2x fp32 MAC rate).
  - ScalarE squares psum with scale 1/64 into fp8 "sq" pairs.
  - Group-of-16 reduction on the PE: fp8 DoubleRow matmul against a
    constant 0/4 block-indicator matrix (S2), accumulating quad directly
    into the per-n-block accumulator psum bank [128k, 512n].
  - Linear term x.Wx (fp8 DoubleRow, same moving x) and x^2.Wxx (fp16 for
    precision -- fp8 wxx/xsq triples the output error) accumulate into the
    same bank, placed before the block's final group-sum so the PE never
    waits on the scalar engine.
  - DVE adds base (per-partition scalar) while copying acc psum -> SBUF
    fp16, DMA out per block (overlapped with the next block's matmuls).
    Host transposes [K,N] -> [N,K] and widens to fp32.

Sharding: rows N=16384 split across 8 cores; params replicated.
DMA: all tensors are stored PIECE-MAJOR in DRAM and SBUF so every piece
transfer is contiguous per partition (1-8KB runs; sub-KB strided runs
crawl at ~12GB/s on the DGE rings). Pieces ride the two hardware rings
(sync + scalar) ordered by first-use time; tiny constants and the block
outputs use the gpsimd software ring.
"""

import math

import numpy as np

K, D_FEAT, L_FAC, N = 128, 1024, 16, 16384
N_CORES = 8
N_SHARD = N // N_CORES            # 2048 rows per core
NB = N_SHARD // 512               # 4 moving blocks of 512 rows
J2 = 4                            # DoubleRow contraction chunks (256 each)
JJ = 8                            # fp16 xsq chunks (128 each)
CB = 16                           # 128-col blocks of factor columns
NPAIR = CB // 2                   # S2 pair matmuls per n-block
WPC = 4                           # wq pieces (512 factor cols each)
GCOLS = K * L_FAC                 # 2048 factor columns
SG = 32.0                         # Gw fp8 scale
SQ_SCALE = 1.0 / 64.0             # scalar: sq = (psum/64)^2 = y^2/4
S2_VAL = 4.0                      # un-scales sq in the group-sum matmul

_CACHE = {}


def _get_nc():
    if "nc" in _CACHE:
        return _CACHE["nc"]

    import concourse.bass as bass
    import concourse.tile as tile
    from concourse import bacc, mybir

    f32 = mybir.dt.float32
    f16 = mybir.dt.float16
    f8 = mybir.dt.float8e4
    DR = mybir.MatmulPerfMode.DoubleRow
    nc = bacc.Bacc("TRN2", target_bir_lowering=False, debug=False,
                   num_devices=N_CORES)

    xq = nc.dram_tensor("xq", [128, NB, J2, 2, 512], f8, kind="ExternalInput").ap()
    xsqh = nc.dram_tensor("xsqh", [128, NB, JJ, 512], f16, kind="ExternalInput").ap()
    wq = nc.dram_tensor("wq", [128, WPC, J2, 2, 512], f8, kind="ExternalInput").ap()
    wx = nc.dram_tensor("wx", [128, J2, 2, K], f8, kind="ExternalInput").ap()
    wxxh = nc.dram_tensor("wxxh", [128, JJ, K], f16, kind="ExternalInput").ap()
    s2 = nc.dram_tensor("s2", [128, NPAIR, 2, K], f8, kind="ExternalInput").ap()
    bs = nc.dram_tensor("bs", [128, 1], f32, kind="ExternalInput").ap()
    outT = nc.dram_tensor("outT", [128, N_SHARD], f16, kind="ExternalOutput").ap()

    with tile.TileContext(nc) as tc:
        with (
            tc.tile_pool(name="singles", bufs=1) as singles,
            tc.tile_pool(name="sqpool", bufs=4) as sqpool,
            tc.tile_pool(name="upool", bufs=2) as upool,
            tc.tile_pool(name="qp", bufs=3, space="PSUM") as qp,
            tc.tile_pool(name="accp", bufs=2, space="PSUM") as accp,
        ):
            wq_s = singles.tile([128, WPC, J2, 2, 512], f8, tag="wq")
            wx_s = singles.tile([128, J2, 2, K], f8, tag="wx")
            xq_s = singles.tile([128, NB, J2, 2, 512], f8, tag="xq")
            xs_s = singles.tile([128, NB, JJ, 512], f16, tag="xs")
            wxx_s = singles.tile([128, JJ, K], f16, tag="wxx")
            s2_s = singles.tile([128, NPAIR, 2, K], f8, tag="s2")
            bs_s = singles.tile([128, 1], f32, tag="bs")
            dmy = singles.tile([128, 2, 512], f8, tag="dmy")

            # gpsimd finishes its framework memsets ~0.2us in and carries no
            # input DMAs, so dmy is ready ~1.3us in (vector would be ~2.9).
            nc.gpsimd.memset(dmy, 0)

            # Ring plan: the SYNC hw ring is by far the fastest (~200+GB/s
            # with >=4KB runs) -- all inputs ride it in first-use order,
            # exactly like the proven baseline schedule. The scalar queue
            # carries no input DMAs (a DMA issue waiting on a recycled DGE
            # semaphore would block the ACTIVATE squares behind it); gpsimd
            # only carries mid-stream block outputs.
            nc.sync.dma_start(out=wq_s[:, 0], in_=wq[:, 0])
            nc.sync.dma_start(out=xq_s[:, 0], in_=xq[:, 0])
            nc.sync.dma_start(out=s2_s, in_=s2)
            nc.sync.dma_start(out=wq_s[:, 1], in_=wq[:, 1])
            nc.sync.dma_start(out=wq_s[:, 2], in_=wq[:, 2])
            nc.sync.dma_start(out=xq_s[:, 1], in_=xq[:, 1])
            nc.sync.dma_start(out=wq_s[:, 3], in_=wq[:, 3])
            nc.sync.dma_start(out=wx_s, in_=wx)
            nc.sync.dma_start(out=bs_s, in_=bs)
            nc.sync.dma_start(out=wxx_s, in_=wxxh)
            nc.sync.dma_start(out=xs_s[:, 0], in_=xsqh[:, 0])
            nc.sync.dma_start(out=xq_s[:, 2], in_=xq[:, 2])
            nc.sync.dma_start(out=xs_s[:, 1], in_=xsqh[:, 1])
            nc.sync.dma_start(out=xq_s[:, 3], in_=xq[:, 3])
            nc.sync.dma_start(out=xs_s[:, 2], in_=xsqh[:, 2])
            nc.sync.dma_start(out=xs_s[:, 3], in_=xsqh[:, 3])

            # Dummy matmuls bridge the PE continuously until the first real
            # data lands: an idle gap would reset the p-state ramp and the
            # first ~14 real matmuls would run at half clock (~4us lost).
            for _ in range(12):
                wrm = qp.tile([128, 2, 512], f32, tag="q")
                nc.tensor.matmul(wrm[:, 0], dmy[:, :, 0:128], dmy,
                                 start=True, stop=True, perf_mode=DR)

            state = {}

            def emit_pairs(nb, pairs):
                st = state.setdefault(nb, {"acc": None, "pending": []})
                if st["acc"] is None:
                    acc_t = accp.tile([128, 512], f32, tag="acc")
                    st["acc"] = acc_t
                acc = st["acc"]
                for pair in pairs:
                    sq_t = sqpool.tile([128, 2, 512], f8, tag="sq")
                    # both halves of the pair share one 2-bank psum tile so
                    # a single double-width ACTIVATE squares the whole pair
                    q = qp.tile([128, 2, 512], f32, tag="q")
                    for r in range(2):
                        cb = 2 * pair + r
                        for j2 in range(J2):
                            nc.tensor.matmul(
                                q[:, r],
                                wq_s[:, cb // 4, j2, :,
                                     (cb % 4) * 128:(cb % 4 + 1) * 128],
                                xq_s[:, nb, j2],
                                start=(j2 == 0), stop=(j2 == J2 - 1),
                                perf_mode=DR)
                    nc.scalar.activation(
                        sq_t, q,
                        mybir.ActivationFunctionType.Square,
                        scale=SQ_SCALE)
                    # defer each group-sum TWO pairs so a late square never
                    # stalls the PE
                    st["pending"].append((pair, sq_t))
                    if len(st["pending"]) > 2:
                        p_pair, p_sq = st["pending"].pop(0)
                        nc.tensor.matmul(acc, s2_s[:, p_pair], p_sq,
                                         start=(p_pair == 0), stop=False,
                                         perf_mode=DR)

            def emit_block_tail(nb, split_out):
                st = state[nb]
                acc = st["acc"]
                nbs = slice(nb * 512, (nb + 1) * 512)
                # linear terms x.Wx and x^2.Wxx run while the scalar engine
                # finishes the last pair's squares
                for j2 in range(J2):
                    nc.tensor.matmul(acc, wx_s[:, j2], xq_s[:, nb, j2],
                                     start=False, stop=False, perf_mode=DR)
                for jj in range(JJ):
                    nc.tensor.matmul(acc, wxx_s[:, jj], xs_s[:, nb, jj],
                                     start=False, stop=False)
                pend = st["pending"]
                for k, (p_pair, p_sq) in enumerate(pend):
                    nc.tensor.matmul(acc, s2_s[:, p_pair], p_sq,
                                     start=False, stop=(k == len(pend) - 1),
                                     perf_mode=DR)
                pend.clear()
                # block output immediately: overlaps later matmuls. The last
                # block goes out in column halves on the (by now idle)
                # scalar ring so the transfer overlaps the second DVE add.
                if not split_out:
                    u = upool.tile([128, 512], f16, tag="u")
                    nc.vector.tensor_scalar_add(out=u, in0=acc, scalar1=bs_s)
                    nc.gpsimd.dma_start(out=outT[:, nbs], in_=u)
                else:
                    u = upool.tile([128, 512], f16, tag="u")
                    nc.vector.tensor_scalar_add(out=u[:, 0:256],
                                                in0=acc[:, 0:256],
                                                scalar1=bs_s)
                    nc.scalar.dma_start(
                        out=outT[:, nb * 512:nb * 512 + 256], in_=u[:, 0:256])
                    nc.vector.tensor_scalar_add(out=u[:, 256:512],
                                                in0=acc[:, 256:512],
                                                scalar1=bs_s)
                    nc.scalar.dma_start(
                        out=outT[:, nb * 512 + 256:(nb + 1) * 512],
                        in_=u[:, 256:512])

            for nb in range(NB):
                emit_pairs(nb, range(NPAIR))
                emit_block_tail(nb, split_out=(nb == NB - 1))

    nc.finalize()
    _CACHE["nc"] = nc
    return nc


def _host_params(PI, MU, A, D):
    import ml_dtypes
    FP8 = ml_dtypes.float8_e4m3

    PI64 = PI.astype(np.float64)
    MU64 = MU.astype(np.float64)
    A64 = A.astype(np.float64)
    D64 = D.astype(np.float64)

    iD = D64 ** -2.0                                   # (K, d)
    iDA = iD[:, :, None] * A64                         # (K, d, l)
    Lm = np.eye(L_FAC)[None] + np.einsum("kdl,kdm->klm", A64, iDA)
    iL = np.linalg.inv(Lm)
    C = np.linalg.cholesky(iL)                         # iL = C C^T
    s = 1.0 / math.sqrt(2.0)
    G = np.einsum("kdl,klm->kdm", iDA, C) * s          # (K, d, l)
    b = np.einsum("kd,kdl->kl", MU64, iDA)             # (K, l)
    h = np.einsum("kl,klm->km", b, C) * s              # (K, l)

    Gw = G.transpose(1, 0, 2).reshape(D_FEAT, GCOLS)   # col k*16+m
    Wx = (iD * MU64).T - 2.0 * np.einsum("kdm,km->kd", G, h).T
    Wxx = -0.5 * iD.T                                  # (d, K)

    det_L = np.linalg.slogdet(Lm)[1]
    log_det_sigma = det_L - np.sum(np.log(iD), axis=1)
    c1 = np.sum(iD * MU64 * MU64, axis=1)
    hsq = np.sum(h * h, axis=1)
    base = PI64 - 0.5 * (D_FEAT * math.log(2.0 * math.pi)
                         + log_det_sigma + c1) + hsq

    # wq piece-major: [128, WPC, J2, 2, 512], piece pc = factor cols
    # [pc*512, (pc+1)*512); d index = j2*256 + r*128 + p.
    gw8 = (Gw * SG).astype(np.float32).astype(FP8)     # (d, 2048)
    wq = np.ascontiguousarray(
        gw8.reshape(J2, 2, 128, WPC, 512).transpose(2, 3, 0, 1, 4))
    wx8 = np.ascontiguousarray(
        Wx.astype(np.float32).astype(FP8)
        .reshape(J2, 2, 128, K).transpose(2, 0, 1, 3))

    wxxh = np.ascontiguousarray(
        Wxx.astype(np.float32).reshape(JJ, 128, K).transpose(1, 0, 2)
    ).astype(np.float16)

    s2 = np.zeros((128, NPAIR, 2, K), dtype=np.float32)
    p_idx = np.arange(128)
    for pair in range(NPAIR):
        for r in range(2):
            cb = 2 * pair + r
            s2[p_idx, pair, r, cb * 8 + p_idx // 16] = S2_VAL
    s2 = s2.astype(FP8)

    bs_np = np.ascontiguousarray(base.astype(np.float32).reshape(128, 1))
    return wq, wx8, wxxh, s2, bs_np


def kernel(x, PI, MU, A, D, _trace=False):
    from concourse.bass_utils import run_bass_kernel_spmd
    import ml_dtypes
    FP8 = ml_dtypes.float8_e4m3

    x = np.asarray(x, dtype=np.float32)
    wq, wx8, wxxh, s2, bs_np = _host_params(
        np.asarray(PI), np.asarray(MU), np.asarray(A), np.asarray(D))

    in_maps = []
    for c in range(N_CORES):
        xs = x[c * N_SHARD:(c + 1) * N_SHARD]          # (2048, 1024)
        x8t = xs.astype(FP8).T                         # (1024, 2048)
        # piece-major: [128, NB, J2, 2, 512]; d = j2*256 + r*128 + p
        xq = np.ascontiguousarray(
            x8t.reshape(J2, 2, 128, NB, 512).transpose(2, 3, 0, 1, 4))
        xsqt = (xs * xs).astype(np.float16).T          # (1024, 2048)
        xsqh = np.ascontiguousarray(
            xsqt.reshape(JJ, 128, NB, 512).transpose(1, 2, 0, 3))
        in_maps.append({
            "xq": xq,
            "xsqh": xsqh,
            "wq": wq,
            "wx": wx8,
            "wxxh": wxxh,
            "s2": s2,
            "bs": bs_np,
        })

    nc = _get_nc()
    res = run_bass_kernel_spmd(nc, in_maps, list(range(N_CORES)),
                               trace=_trace)
    _CACHE["last_results"] = res
    outT = np.concatenate([res.results[c]["outT"] for c in range(N_CORES)],
                          axis=1)                      # (128, 16384)
    return np.ascontiguousarray(outT.T).astype(np.float32)
